# revision 15
# baseline (speedup 1.0000x reference)
"""SAGAN-style self-attention block on 8 Trainium2 NeuronCores.

Reference computation (per batch element b, C=128, H=W=64, N=4096):
    theta = W_theta @ x_b                       [16, 4096]
    phi   = maxpool2x2(W_phi @ x_b)             [16, 1024]
    g     = maxpool2x2(W_g @ x_b)               [64, 1024]
    S     = theta^T phi                         [4096, 1024]
    beta  = softmax(S, axis=-1)
    o     = g @ beta^T                          [64, 4096]
    out   = gamma * (W_o @ o) + x_b             [128, 4096]

Sharding: data-parallel over batch; core b gets batch element b; weights
replicated; no collectives.

Device dataflow (computes S^T = phi^T theta so softmax's reduction axis
lands on the PE contraction axis; row-sums come for free from a ones
column folded into the g^T stationary operand):

  Conv (per body: 8 x-chunks of 512 through one PSUM bank):
    cc [112, 512] = wgp^T @ x_chunk  (wgp = [W_g | W_phi | 0 | W_phi])
    maxpool tensor_reduce -> pg[:, 128c]; phi for quadrants 0/1 copied
    by SBUF->SBUF DMA (scalar queue); g^T via PE transpose into a
    col-slice of the conv bank + DVE copy into gTa.
    The conv for body u+1 is WOVEN into body u's attention chunk loop
    (one conv chunk per attention chunk) against ping-pong pg/phi/gTa
    buffers, so its serial MM->reduce->transpose->copy chain hides
    completely under the ACT-paced attention.  Requires even UNROLL for
    slot parity across the For_i back edge; body UNROLL-1 reloads input
    slot 0 at its top so the woven conv reads next-iteration data.

  Attention loop over 8 n-chunks of 512 (ACT exp stream is the pacer):
    theta [112, 512]   conv, software-pipelined one chunk ahead
    S^T   [128m, 512n] = phi_q^T theta_q, 4 concurrent K=16 matmuls via
                         tile_position row groups
    E^T = exp(S^T)     ACT, bf16 out (no max subtraction: |S| <= ~12)
    po [128, 512] = sum_m gTa_m^T @ E^T_m -> row 0 = s_n (softmax
                    denominator), rows 64..127 = unnormalized o
    rcp = 1/s on the [1,512] sums row; broadcast to 128 partitions by
    the GPSIMD partition_broadcast Q7 op (replaces the baseline
    ones-matmul + DVE copy + f32r cast)
    po2 = (gamma*W_o) @ o  (gamma folded into the weight host-side)
    out = po2 * rcp_bcast + x   (DVE stt + GPSIMD add; the residual
          reads the f32r x upload bitcast as f32 -- saves a second
          2MB HBM read, costing ~1e-4 relative error)

  HAM keep-warm: the PE clock-gate (K=4/8 at 1.2 GHz when the HAM sees
  idle windows) was measured cold for ~85% of the baseline run, doubling
  every matmul.  Dependency stalls are unavoidable (ACT paces each chunk
  at ~4us vs ~3.3us of PE work), so wait-free full-array heater matmuls
  (K=128 x M=128 x N=H_N from a constant SBUF tile into the conv bank)
  fill the stall sites.  K=1 heaters do NOT work: HAM watches array
  activity, not instruction occupancy.

Matmul operands use the FP32R format (fp32 with mantissa rounded to 11
bits; full-rate PE streaming vs 1/4-rate fp32). Host inputs are
pre-rounded; on-device producers write float32r APs so the engines round
on the write port (walrus checkMatmultFP32r requires rounded producers).
"""

import os
import numpy as np

MM_MODE = os.environ.get("K_MM_MODE", "f32r")  # f32r | f32
ET_BF16 = os.environ.get("K_ET_BF16", "1") == "1"  # bf16 attention weights
# heater tuning knobs (count of full-array N=H_N heater matmuls per site)
H_N = int(os.environ.get("K_HEAT_N", "256"))      # heater free dim
H_PRE = int(os.environ.get("K_HEAT_PRE", "2"))    # top of chunk (exp drain)
H_MID = int(os.environ.get("K_HEAT_MID", "1"))    # after thp pipelining
H_PO = int(os.environ.get("K_HEAT_PO", "1"))      # between po pairs
H_CONV = int(os.environ.get("K_HEAT_CONV", "2"))  # prologue conv steps
N_CORES = 8
C = 128
N = 4096       # H*W
M = 1024       # N/4
NCH = 8        # n-chunks
CHUNK = 512


def _round_fp32r(a: np.ndarray) -> np.ndarray:
    """Round fp32 to the FP32R grid (11-bit mantissa, round-half-even)."""
    u = np.ascontiguousarray(a, dtype=np.float32).view(np.uint32)
    lsb = (u >> np.uint32(12)) & np.uint32(1)
    r = (u + np.uint32(0x7FF) + lsb) & np.uint32(0xFFFFF000)
    return r.view(np.float32)


def _build(reps: int = 1):
    from contextlib import nullcontext
    import concourse.bass as bass
    import concourse.tile as tile
    from concourse import bacc, mybir

    f32 = mybir.dt.float32
    fmm = mybir.dt.float32r if MM_MODE == "f32r" else f32
    fet = mybir.dt.bfloat16 if ET_BF16 else fmm
    ts = bass.ts
    ALU = mybir.AluOpType
    ACTF = mybir.ActivationFunctionType

    nc = bacc.Bacc(
        "TRN2", target_bir_lowering=False, debug=False, enable_asserts=False,
        num_devices=N_CORES,
    )

    # The GPSIMD library-load pass greedily alternates between `standard`
    # (tensor_tensor) and `attn` (partition_broadcast), paying a hidden
    # ~6us Q7 IRAM reload per switch -- 2x per chunk.  Both instructions
    # live in the `proxy` library (built for exactly this no-mid-kernel-
    # reload case), so restrict their masks to proxy: the pass then emits
    # ONE standard->proxy load at kernel start and never reloads.
    import types
    import bass_rust as _br
    import concourse.bass_isa as _bisa
    import concourse.library_config as _lc

    def _insert_library_loads_proxy(self):
        if not _lc.check_generated_files():
            raise RuntimeError("library config out of date")
        proxy_bit = 1 << _lc.proxy.index
        m: dict = {}
        for lib in _lc.all_libraries:
            for it in lib.instructions:
                m[it] = m.get(it, 0) | (1 << lib.index)
        m[mybir.InstTensorTensor] = proxy_bit
        m[_bisa.InstPartitionBroadcast] = proxy_bit
        _br.insert_library_loads(self, m, len(_lc.all_libraries),
                                 _lc.standard.index)

    nc.insert_library_loads = types.MethodType(_insert_library_loads_proxy, nc)
    xr_d = nc.dram_tensor("xr", [C, N], fmm, kind="ExternalInput")
    # all matmul weights packed in one DMA:
    #   cols 0:112   wt_rep  (W_theta^T replicated at quadrant offsets)
    #   cols 112:224 wgp     ([W_g | W_phi | 0 | W_phi] -> conv rows:
    #                         g 0:64, phi 64:80, junk 80:96, phi 96:112)
    #   cols 224:352 wo_t    (rows 64:128 = (gamma*W_o)^T)
    wcat_d = nc.dram_tensor("wcat", [128, 352], fmm, kind="ExternalInput")
    id_d = nc.dram_tensor("ident", [64, 64], fmm, kind="ExternalInput")
    out_d = nc.dram_tensor("out", [C, N], f32, kind="ExternalOutput")

    # even UNROLL required for conv-weave slot parity across the back edge
    UNROLL = 1
    if reps > 1:
        for d in (6, 4, 2):
            if reps % d == 0:
                UNROLL = d
                break
    WEAVE = UNROLL % 2 == 0 and UNROLL > 1

    with tile.TileContext(nc) as tc:
        with (
            tc.tile_pool(name="persist", bufs=1) as persist,
            tc.tile_pool(name="theta", bufs=2) as thpool,
            tc.tile_pool(name="et", bufs=8) as etp,
            tc.tile_pool(name="work", bufs=2) as work,
            tc.tile_pool(name="outp", bufs=3) as outpool,
            # PSUM bank budget (8 banks):
            #   pspair 2 slots x [128,1024] = 4   S^T pair tiles
            #   psth   1                          theta conv
            #   psacc  1                          po accumulator
            #   pssm   1                          po2
            #   psconv 1                          conv chunk + ptr + heaters
            tc.tile_pool(name="pspair", bufs=2, space="PSUM") as pspair,
            tc.tile_pool(name="psth", bufs=1, space="PSUM") as psth,
            tc.tile_pool(name="psacc", bufs=1, space="PSUM") as psacc,
            tc.tile_pool(name="pssm", bufs=1, space="PSUM") as pssm,
            tc.tile_pool(name="psconv", bufs=1, space="PSUM") as psconv,
        ):
          # ---- loop-invariant constants ---------------------------------
          heatw_f = persist.tile([128, 512], f32, name="heatw_f")
          nc.vector.memset(heatw_f, 0.5)
          heatw = persist.tile([128, 512], fet, name="heatw")
          nc.vector.tensor_copy(heatw, heatw_f)
          # ping-pong conv outputs (slot = body index % 2)
          gTa_s, pg_s, ph_s = [], [], []
          for s in range(2):
              gTa_t = persist.tile([128, 8 * 128], fet, name=f"gTa{s}")
              nc.vector.memset(gTa_t, 0.0)
              nc.vector.memset(
                  gTa_t[:, :].rearrange("p (b c) -> p b c", c=128)[:, :, 0:1], 1.0
              )
              gTa_s.append(gTa_t)
              pg_s.append(persist.tile([112, M], fmm, name=f"pg{s}"))
              ph_s.append(persist.tile([48, M], fmm, name=f"ph{s}"))
          id_sb = persist.tile([64, 64], fmm, name="id_sb")
          nc.sync.dma_start(id_sb, id_d[:, :])

          NSLOT = max(UNROLL, 1)
          xr_tiles = [persist.tile([C, N], fmm, name=f"XrS{i}") for i in range(NSLOT)]
          wcat_tiles = [
              persist.tile([128, 352], fmm, name=f"wcatS{i}") for i in range(NSLOT)
          ]

          def load_inputs(slot):
              # weights first: the first conv matmul needs wcat
              nc.sync.dma_start(wcat_tiles[slot], wcat_d[:, :])
              for k in range(NCH):
                  nc.sync.dma_start(
                      xr_tiles[slot][:, ts(k, CHUNK)], xr_d[:, ts(k, CHUNK)]
                  )

          # conv bank: [128, 512] f32; conv matmul uses rows 0:112, PE
          # transposes park g^T in cols 448:512 between the reduce and
          # the gTa copy, and heaters write [0:128, 0:H_N] after the
          # reduce has consumed the data.  Allocated once per For_i
          # iteration (persistent slot) -- all uses serialize through
          # Tile's bank-aware tracker.
          cvb_box = [None]

          def heat(k):
              # full-array keep-warm matmuls (K=128 x M=128 x N=H_N):
              # HAM watches array activity, so thin matmuls don't count
              cvb = cvb_box[0]
              for _ in range(k):
                  nc.tensor.matmul(
                      cvb[:, 0:H_N], heatw[:, 0:128], heatw[:, 0:H_N],
                      start=True, stop=True,
                  )

          def conv_mm(p, Xr_n, wgp_n, pg, ph):
              # conv matmul + maxpool reduce + phi DMAs for one x-chunk
              cvb = cvb_box[0]
              nc.tensor.matmul(
                  cvb[0:112, :], wgp_n, Xr_n[:, ts(p, CHUNK)],
                  start=True, stop=True,
              )
              nc.vector.tensor_reduce(
                  out=pg[:, ts(p, 128)].rearrange("p (i j) -> p i j", i=4, j=32),
                  in_=cvb[0:112, :].rearrange(
                      "p (i di j dj) -> p i j di dj", i=4, di=2, j=32, dj=2
                  ),
                  axis=mybir.AxisListType.XY,
                  op=ALU.max,
              )
              # phi for quadrants 0/1: SBUF->SBUF DMA (scalar queue)
              nc.scalar.dma_start(ph[0:16, ts(p, 128)], pg[64:80, ts(p, 128)])
              nc.scalar.dma_start(ph[32:48, ts(p, 128)], pg[64:80, ts(p, 128)])

          def conv_tr(p, pg, gTa_t):
              # g^T via PE transpose into the conv bank's tail columns;
              # emitted in a PE-slack region (the reduce is long done, so
              # the transpose never blocks the FIFO head)
              cvb = cvb_box[0]
              nc.tensor.transpose(
                  cvb[:, 448:512].bitcast(fmm), pg[0:64, ts(p, 128)], id_sb
              )
              nc.vector.tensor_copy(
                  gTa_t[:, p * 128 + 64 : p * 128 + 128],
                  cvb[:, 448:512].bitcast(fmm),
              )

          def conv_step(p, Xr_n, wgp_n, pg, ph, gTa_t):
              conv_mm(p, Xr_n, wgp_n, pg, ph)
              conv_tr(p, pg, gTa_t)

          def body(u):
              """Attention for body u (+ woven conv for body u+1)."""
              s = u % 2
              pg, ph, gTa_t = pg_s[s], ph_s[s], gTa_s[s]
              Xr = xr_tiles[u % NSLOT]
              wcat = wcat_tiles[u % NSLOT]
              Xf = Xr.bitcast(f32)
              wt_sb = wcat[:, 0:112]
              wo_sb = wcat[:, 224:352]
              if WEAVE:
                  ns = (u + 1) % 2
                  Xr_n = xr_tiles[(u + 1) % NSLOT]
                  wgp_n = wcat_tiles[(u + 1) % NSLOT][:, 112:224]
                  pg_n, ph_n, gTa_n = pg_s[ns], ph_s[ns], gTa_s[ns]

              def phi_q(j, mi):
                  blk = ts(mi, 128)
                  if j == 0:
                      return ph[0:16, blk]
                  if j == 1:
                      return ph[32:48, blk]
                  if j == 2:
                      return pg[64:80, blk]
                  return pg[96:112, blk]

              outp_box = [None]

              def emit_theta(ci):
                  thp = psth.tile([128, CHUNK], f32, name="thp", tag="th")
                  nc.tensor.matmul(
                      thp[0:112, :], wt_sb, Xr[:, ts(ci, CHUNK)],
                      start=True, stop=True,
                  )
                  theta = thpool.tile([112, CHUNK], fmm, name="theta", tag="theta")
                  nc.vector.tensor_copy(theta, thp[0:112, :])
                  return theta

              def emit_po2(st):
                  ci, o_sb, rbc = st
                  po2 = pssm.tile([128, CHUNK], f32, name="po2", tag="small")
                  nc.tensor.matmul(
                      po2,
                      wo_sb[64:128, :],
                      o_sb[64:128, :],
                      start=True,
                      stop=True,
                      tile_position=(64, 0),
                  )
                  return po2

              def emit_tail(st, po2):
                  # normalize + residual for chunk ci (rbc broadcast has
                  # been in flight for a full chunk)
                  ci, o_sb, rbc = st
                  t1 = work.tile([128, CHUNK], f32, name="t1", tag="t1")
                  nc.vector.scalar_tensor_tensor(
                      t1, in0=po2, scalar=1.0, in1=rbc,
                      op0=ALU.mult, op1=ALU.mult,
                  )
                  if ci % 2 == 0:
                      outp_box[0] = outpool.tile(
                          [128, 2 * CHUNK], f32, name="outp", tag="out"
                      )
                  outp = outp_box[0]
                  half = outp[:, (ci % 2) * CHUNK : (ci % 2) * CHUNK + CHUNK]
                  nc.gpsimd.tensor_add(half, t1, Xf[:, ts(ci, CHUNK)])
                  if ci % 2 == 1:
                      # store via the GPSIMD SWDGE queue: keeps the SP
                      # HWDGE ring free for input loads
                      nc.gpsimd.dma_start(
                          out_d[:, bass.ds((ci - 1) * CHUNK, 2 * CHUNK)], outp
                      )

              def emit_sq(q, theta_t):
                  # one S^T quadrant-group: 4 concurrent K=16 matmuls
                  pair_a = pspair.tile([128, 1024], f32, name="pair_a", tag="pair")
                  pair_b = pspair.tile([128, 1024], f32, name="pair_b", tag="pair")
                  for j in range(4):
                      mi = 4 * q + j
                      dst = (pair_a if j < 2 else pair_b)[
                          :, (j % 2) * CHUNK : (j % 2) * CHUNK + CHUNK
                      ]
                      nc.tensor.matmul(
                          dst,
                          phi_q(j, mi),
                          theta_t[32 * j : 32 * j + 16, :],
                          start=True,
                          stop=True,
                          tile_position=(32 * j, 0),
                      )
                  return (pair_a, pair_b)

              def emit_exps(pairs, ets):
                  for pair in pairs:
                      et = etp.tile([128, 1024], fet, name="et", tag="et")
                      nc.scalar.activation(et, pair, ACTF.Exp)
                      ets.append(et)

              # Steady-state chunk emission keeps the ACT exp stream
              # gap-free: S^T(ci) q1 is emitted right after q0's exps
              # (only cheap wait-free PE work between them), and
              # S^T(ci+1) q0 is woven into the po stream at the exact
              # point where its slot-recycling wait (exp(ci) q1a) fires.
              pending = None
              theta = emit_theta(0)
              pairs_q0 = emit_sq(0, theta)
              for ci in range(NCH):
                  ets = []
                  po2_prev = None
                  heat(H_PRE)
                  emit_exps(pairs_q0, ets)
                  if pending is not None:
                      po2_prev = emit_po2(pending)
                  if WEAVE and ci >= 1:
                      conv_mm(ci - 1, Xr_n, wgp_n, pg_n, ph_n)
                  theta_nxt = emit_theta(ci + 1) if ci + 1 < NCH else None
                  pairs_q1 = emit_sq(1, theta)
                  emit_exps(pairs_q1, ets)
                  theta = theta_nxt
                  heat(H_MID)

                  po = psacc.tile([128, CHUNK], f32, name="po", tag="acc")
                  for mi in range(4):
                      rhs = ets[mi // 2][:, (mi % 2) * CHUNK : (mi % 2) * CHUNK + CHUNK]
                      nc.tensor.matmul(
                          po, gTa_t[:, mi * 128 : (mi + 1) * 128], rhs,
                          start=(mi == 0), stop=False,
                      )
                  if ci + 1 < NCH:
                      pairs_q0 = emit_sq(0, theta)
                  for mi in range(4, 8):
                      rhs = ets[mi // 2][:, (mi % 2) * CHUNK : (mi % 2) * CHUNK + CHUNK]
                      nc.tensor.matmul(
                          po, gTa_t[:, mi * 128 : (mi + 1) * 128], rhs,
                          start=False, stop=(mi == 7),
                      )
                  if WEAVE and ci >= 1:
                      conv_tr(ci - 1, pg_n, gTa_n)
                  heat(H_PO)

                  if pending is not None:
                      emit_tail(pending, po2_prev)
                      pending = None

                  # full-height po copy brings the s row (row 0) and o
                  # (rows 64:127) to SBUF; reciprocal of s and its Q7
                  # broadcast launch here so the tail streams next chunk
                  o_sb = work.tile([128, CHUNK], fmm, name="o_sb", tag="osb")
                  nc.vector.tensor_copy(o_sb, po)
                  rcp_f = work.tile([1, CHUNK], f32, name="rcp_f", tag="rcpf")
                  nc.vector.reciprocal_approx_fast(rcp_f, o_sb[0:1, :].bitcast(f32))
                  rbc = work.tile([128, CHUNK], f32, name="rbc", tag="rbc")
                  nc.gpsimd.partition_broadcast(rbc[:, :], rcp_f[0:1, :])

                  pending = (ci, o_sb, rbc)
              po2_last = emit_po2(pending)
              if WEAVE:
                  conv_step(NCH - 1, Xr_n, wgp_n, pg_n, ph_n, gTa_n)
              emit_tail(pending, po2_last)

          # ---- prologue: first input + body-0 conv (serial, once) -------
          load_inputs(0)
          cvb_box[0] = psconv.tile([128, CHUNK], f32, name="cvb", tag="cv")
          for p in range(NCH):
              conv_step(p, xr_tiles[0], wcat_tiles[0][:, 112:224],
                        pg_s[0], ph_s[0], gTa_s[0])
              heat(H_CONV)

          loop_cm = (
              tc.For_i(
                  0, reps // UNROLL, 1,
                  hint_engines=(
                      mybir.EngineType.PE,
                      mybir.EngineType.DVE,
                      mybir.EngineType.Activation,
                  ),
              )
              if reps > 1
              else nullcontext()
          )
          with loop_cm:
           for _s in range(1, UNROLL):
               load_inputs(_s)
           cvb_box[0] = psconv.tile([128, CHUNK], f32, name="cvb", tag="cv")
           for _u in range(UNROLL):
               if _u == UNROLL - 1 and UNROLL > 1:
                   # reload slot 0 BEFORE the last body so its woven conv
                   # (for next iteration's body 0) reads fresh data
                   load_inputs(0)
               if not WEAVE and _u > 0:
                   # fallback: serial conv at body top (odd UNROLL)
                   for p in range(NCH):
                       conv_step(p, xr_tiles[_u % NSLOT],
                                 wcat_tiles[_u % NSLOT][:, 112:224],
                                 pg_s[_u % 2], ph_s[_u % 2], gTa_s[_u % 2])
               body(_u)
           if UNROLL == 1 and reps > 1:
               # single-body loop: conv for the next iteration (slot 0)
               load_inputs(0)
               for p in range(NCH):
                   conv_step(p, xr_tiles[0], wcat_tiles[0][:, 112:224],
                             pg_s[0], ph_s[0], gTa_s[0])

    nc.compile()
    return nc


def _host_prep(x, W_theta, W_phi, W_g, W_o, gamma):
    B = np.asarray(x).shape[0]
    rnd = _round_fp32r if MM_MODE == "f32r" else (lambda a: np.asarray(a, np.float32))
    wcat = np.zeros((128, 352), dtype=np.float32)
    for j in range(4):
        wcat[:, 32 * j : 32 * j + 16] = np.asarray(W_theta, np.float32).T
    wcat[:, 112:176] = np.asarray(W_g, np.float32).T
    wcat[:, 176:192] = np.asarray(W_phi, np.float32).T
    wcat[:, 208:224] = np.asarray(W_phi, np.float32).T
    wcat[64:128, 224:352] = float(gamma) * np.asarray(W_o, np.float32).T
    wcat = rnd(wcat)
    ident = np.eye(64, dtype=np.float32)
    xr = rnd(np.ascontiguousarray(np.asarray(x, dtype=np.float32)))
    in_maps = []
    for b in range(B):
        in_maps.append(
            {
                "xr": np.ascontiguousarray(xr[b].reshape(C, N)),
                "wcat": wcat,
                "ident": ident,
            }
        )
    return in_maps


def run(x, W_theta, W_phi, W_g, W_o, gamma, trace=False, **trace_kwargs):
    from concourse.bass_utils import run_bass_kernel_spmd

    nc = _build()
    in_maps = _host_prep(x, W_theta, W_phi, W_g, W_o, gamma)
    res = run_bass_kernel_spmd(
        nc, in_maps, core_ids=list(range(N_CORES)), trace=trace, **trace_kwargs
    )
    outs = [res.results[b]["out"].reshape(C, 64, 64) for b in range(N_CORES)]
    return np.stack(outs).astype(np.float32), res


def kernel(x, W_theta, W_phi, W_g, W_o, gamma):
    out, _ = run(x, W_theta, W_phi, W_g, W_o, gamma)
    return out


# revision 17
# speedup vs baseline: 1.0112x; 1.0112x over previous
"""SAGAN-style self-attention block on 8 Trainium2 NeuronCores.

Reference computation (per batch element b, C=128, H=W=64, N=4096):
    theta = W_theta @ x_b                       [16, 4096]
    phi   = maxpool2x2(W_phi @ x_b)             [16, 1024]
    g     = maxpool2x2(W_g @ x_b)               [64, 1024]
    S     = theta^T phi                         [4096, 1024]
    beta  = softmax(S, axis=-1)
    o     = g @ beta^T                          [64, 4096]
    out   = gamma * (W_o @ o) + x_b             [128, 4096]

Sharding: data-parallel over batch; core b gets batch element b; weights
replicated; no collectives.

Device dataflow.  S^T is computed by associativity as (W_theta^T phi)^T
x: the conv phase precomputes M_mi = W_theta^T @ phi_mi [128c, 128m] per
m-block, so the attention loop's S^T matmuls read the raw input x
directly (no theta conv, no PSUM->SBUF theta cast, and the S^T -> exp
chain depends on nothing but PE + pair-slot recycling).  Softmax row
sums come for free from a ones column folded into the g^T stationary
operand of the po matmul.

  Conv weave (per body: 8 x-chunks of 512 through one PSUM bank):
    cc [80, 512] = wgp^T @ x_chunk   (wgp = [W_g | W_phi])
    maxpool tensor_reduce -> pg[:, 128c]   (g rows 0:64, phi rows 64:80)
    M_c  = W_theta^T @ phi_c  (PE, into the conv bank after the reduce)
    g^T via PE transpose into a col-slice of the conv bank
    DVE copies move M_c -> Msb and g^T -> gTa.
    The conv for body u+1 is WOVEN into body u's attention chunk loop
    (one conv chunk per attention chunk) against ping-pong pg/Msb/gTa
    buffers, so its serial chain hides under the ACT-paced attention.
    Requires even UNROLL for slot parity across the For_i back edge;
    body UNROLL-1 reloads input slot 0 at its top so the woven conv
    reads next-iteration data.

  Attention loop over 8 n-chunks of 512 (ACT exp stream is the pacer):
    S^T   [128m, 512n] = M_mi^T x_chunk, 8 K=128 matmuls into 3
          rotating PSUM pair tiles (6 banks) -- the 3-deep rotation
          gives S^T a full exp of slack vs the slot-recycling wait
    E^T = exp(S^T)     ACT, bf16 out (no max subtraction: |S| <= ~12)
    po [128, 512] = sum_m gTa_m^T @ E^T_m -> row 0 = s_n (softmax
                    denominator), rows 64..127 = unnormalized o
    rcp = 1/s on the [1,512] sums row; broadcast to 128 partitions by
    the GPSIMD partition_broadcast Q7 op
    po2 = (gamma*W_o) @ o, time-shared into the accumulator bank (the
          bank is idle between o_sb evacuation and the next po)
    out = po2 * rcp_bcast + x   (DVE stt + GPSIMD add; the residual
          reads the f32r x upload bitcast as f32 -- saves a second
          2MB HBM read, costing ~1e-4 relative error)

  HAM keep-warm: the PE clock-gate (K=4/8 at 1.2 GHz when the HAM sees
  idle windows) halves matmul speed when dependency gaps pepper the PE
  stream.  Wait-free full-array heater matmuls (K=128 x M=128 x N=H_N
  from a constant SBUF tile into the conv bank) fill the stall sites.
  K=1 heaters do NOT work: HAM watches array activity.

  GPSIMD library: tensor_add and partition_broadcast are pinned to the
  `proxy` Q7 library via an instance-level patch of the library-load
  pass; the default greedy assignment alternates standard/attn, paying
  a hidden ~6us IRAM reload twice per chunk.

Matmul operands use the FP32R format (fp32 with mantissa rounded to 11
bits; full-rate PE streaming vs 1/4-rate fp32). Host inputs are
pre-rounded; on-device producers write float32r APs so the engines round
on the write port (walrus checkMatmultFP32r requires rounded producers).
"""

import os
import numpy as np

MM_MODE = os.environ.get("K_MM_MODE", "f32r")  # f32r | f32
ET_BF16 = os.environ.get("K_ET_BF16", "1") == "1"  # bf16 attention weights
# heater tuning knobs (count of full-array N=H_N heater matmuls per site)
H_N = int(os.environ.get("K_HEAT_N", "256"))      # heater free dim
H_PRE = int(os.environ.get("K_HEAT_PRE", "1"))    # top of chunk
H_MID = int(os.environ.get("K_HEAT_MID", "1"))    # before po
H_PO = int(os.environ.get("K_HEAT_PO", "1"))      # after po
H_CONV = int(os.environ.get("K_HEAT_CONV", "2"))  # prologue conv steps
N_CORES = 8
C = 128
N = 4096       # H*W
M = 1024       # N/4
NCH = 8        # n-chunks
CHUNK = 512


def _round_fp32r(a: np.ndarray) -> np.ndarray:
    """Round fp32 to the FP32R grid (11-bit mantissa, round-half-even)."""
    u = np.ascontiguousarray(a, dtype=np.float32).view(np.uint32)
    lsb = (u >> np.uint32(12)) & np.uint32(1)
    r = (u + np.uint32(0x7FF) + lsb) & np.uint32(0xFFFFF000)
    return r.view(np.float32)


def _build(reps: int = 1):
    from contextlib import nullcontext
    import concourse.bass as bass
    import concourse.tile as tile
    from concourse import bacc, mybir

    f32 = mybir.dt.float32
    fmm = mybir.dt.float32r if MM_MODE == "f32r" else f32
    fet = mybir.dt.bfloat16 if ET_BF16 else fmm
    ts = bass.ts
    ALU = mybir.AluOpType
    ACTF = mybir.ActivationFunctionType

    nc = bacc.Bacc(
        "TRN2", target_bir_lowering=False, debug=False, enable_asserts=False,
        num_devices=N_CORES,
    )

    # Pin the GPSIMD ops to the `proxy` library (see module docstring).
    import types
    import bass_rust as _br
    import concourse.bass_isa as _bisa
    import concourse.library_config as _lc

    def _insert_library_loads_proxy(self):
        if not _lc.check_generated_files():
            raise RuntimeError("library config out of date")
        proxy_bit = 1 << _lc.proxy.index
        m: dict = {}
        for lib in _lc.all_libraries:
            for it in lib.instructions:
                m[it] = m.get(it, 0) | (1 << lib.index)
        m[mybir.InstTensorTensor] = proxy_bit
        m[_bisa.InstPartitionBroadcast] = proxy_bit
        _br.insert_library_loads(self, m, len(_lc.all_libraries),
                                 _lc.standard.index)

    nc.insert_library_loads = types.MethodType(_insert_library_loads_proxy, nc)

    xr_d = nc.dram_tensor("xr", [C, N], fmm, kind="ExternalInput")
    # all matmul weights packed in one DMA:
    #   rows 64:80 cols 0:128   W_theta  (lhsT for the M precompute;
    #                           base partition matches phi rows 64:80)
    #   cols 128:208            wgp = [W_g | W_phi] -> conv rows:
    #                           g 0:64, phi 64:80
    #   cols 208:336            wo_t  (rows 64:128 = (gamma*W_o)^T)
    wcat_d = nc.dram_tensor("wcat", [128, 336], fmm, kind="ExternalInput")
    id_d = nc.dram_tensor("ident", [64, 64], fmm, kind="ExternalInput")
    out_d = nc.dram_tensor("out", [C, N], f32, kind="ExternalOutput")

    # even UNROLL required for conv-weave slot parity across the back edge
    UNROLL = 1
    if reps > 1:
        for d in (6, 4, 2):
            if reps % d == 0:
                UNROLL = d
                break
    WEAVE = UNROLL % 2 == 0 and UNROLL > 1

    with tile.TileContext(nc) as tc:
        with (
            tc.tile_pool(name="persist", bufs=1) as persist,
            tc.tile_pool(name="et", bufs=8) as etp,
            tc.tile_pool(name="work", bufs=2) as work,
            tc.tile_pool(name="outp", bufs=3) as outpool,
            # PSUM bank budget (8 banks):
            #   pspair 3 slots x [128,1024] = 6   S^T pair tiles
            #   psacc  1                          po accumulator + po2
            #   psconv 1                          conv chunk + ptr + M + heat
            tc.tile_pool(name="pspair", bufs=3, space="PSUM") as pspair,
            tc.tile_pool(name="psacc", bufs=1, space="PSUM") as psacc,
            tc.tile_pool(name="psconv", bufs=1, space="PSUM") as psconv,
        ):
          # ---- loop-invariant constants ---------------------------------
          heatw_f = persist.tile([128, 512], f32, name="heatw_f")
          nc.vector.memset(heatw_f, 0.5)
          heatw = persist.tile([128, 512], fet, name="heatw")
          nc.vector.tensor_copy(heatw, heatw_f)
          # ping-pong conv outputs (slot = body index % 2)
          gTa_s, pg_s, msb_s = [], [], []
          for s in range(2):
              gTa_t = persist.tile([128, 8 * 128], fet, name=f"gTa{s}")
              nc.vector.memset(gTa_t, 0.0)
              nc.vector.memset(
                  gTa_t[:, :].rearrange("p (b c) -> p b c", c=128)[:, :, 0:1], 1.0
              )
              gTa_s.append(gTa_t)
              pg_s.append(persist.tile([80, M], fmm, name=f"pg{s}"))
              msb_s.append(persist.tile([128, 8 * 128], fmm, name=f"msb{s}"))
          id_sb = persist.tile([64, 64], fmm, name="id_sb")
          nc.sync.dma_start(id_sb, id_d[:, :])

          NSLOT = max(UNROLL, 1)
          xr_tiles = [persist.tile([C, N], fmm, name=f"XrS{i}") for i in range(NSLOT)]
          wcat_tiles = [
              persist.tile([128, 336], fmm, name=f"wcatS{i}") for i in range(NSLOT)
          ]

          def load_inputs(slot):
              # weights first: the first conv matmul needs wcat
              nc.sync.dma_start(wcat_tiles[slot], wcat_d[:, :])
              for k in range(NCH):
                  nc.sync.dma_start(
                      xr_tiles[slot][:, ts(k, CHUNK)], xr_d[:, ts(k, CHUNK)]
                  )

          # conv bank: [128, 512] f32.  Conv matmul writes rows 0:80;
          # after the reduce consumes it, the same bank hosts (in order)
          # the g^T transpose (cols 448:512), the M_c matmul (cols
          # 0:128), and heater matmuls.  Tile's bank-aware tracker
          # serializes all of it.
          cvb_box = [None]

          def heat(k):
              # full-array keep-warm matmuls (K=128 x M=128 x N=H_N):
              # HAM watches array activity, so thin matmuls don't count
              cvb = cvb_box[0]
              for _ in range(k):
                  nc.tensor.matmul(
                      cvb[:, 0:H_N], heatw[:, 0:128], heatw[:, 0:H_N],
                      start=True, stop=True,
                  )

          def conv_mm(p, Xr_n, wgp_n, pg):
              # conv matmul + maxpool reduce for one x-chunk
              cvb = cvb_box[0]
              nc.tensor.matmul(
                  cvb[0:80, :], wgp_n, Xr_n[:, ts(p, CHUNK)],
                  start=True, stop=True,
              )
              nc.vector.tensor_reduce(
                  out=pg[:, ts(p, 128)].rearrange("p (i j) -> p i j", i=4, j=32),
                  in_=cvb[0:80, :].rearrange(
                      "p (i di j dj) -> p i j di dj", i=4, di=2, j=32, dj=2
                  ),
                  axis=mybir.AxisListType.XY,
                  op=ALU.max,
              )

          def conv_tr(p, wt_n, pg, gTa_t, msb_t):
              # post-reduce conv bank work, emitted in a PE-slack region:
              # g^T transpose + M_c = W_theta^T @ phi_c, plus the DVE
              # copies that move both to SBUF
              cvb = cvb_box[0]
              nc.tensor.transpose(
                  cvb[:, 448:512].bitcast(fmm), pg[0:64, ts(p, 128)], id_sb
              )
              nc.tensor.matmul(
                  cvb[:, 0:128], wt_n, pg[64:80, ts(p, 128)],
                  start=True, stop=True,
              )
              nc.vector.tensor_copy(
                  gTa_t[:, p * 128 + 64 : p * 128 + 128],
                  cvb[:, 448:512].bitcast(fmm),
              )
              nc.vector.tensor_copy(
                  msb_t[:, ts(p, 128)], cvb[:, 0:128].bitcast(fmm)
              )

          def conv_step(p, Xr_n, wt_n, wgp_n, pg, gTa_t, msb_t):
              conv_mm(p, Xr_n, wgp_n, pg)
              conv_tr(p, wt_n, pg, gTa_t, msb_t)

          def body(u):
              """Attention for body u (+ woven conv for body u+1)."""
              s = u % 2
              pg, msb, gTa_t = pg_s[s], msb_s[s], gTa_s[s]
              Xr = xr_tiles[u % NSLOT]
              wcat = wcat_tiles[u % NSLOT]
              Xf = Xr.bitcast(f32)
              wo_sb = wcat[:, 208:336]
              if WEAVE:
                  ns = (u + 1) % 2
                  Xr_n = xr_tiles[(u + 1) % NSLOT]
                  wcat_n = wcat_tiles[(u + 1) % NSLOT]
                  wt_n = wcat_n[64:80, 0:128]
                  wgp_n = wcat_n[:, 128:208]
                  pg_n, msb_n, gTa_n = pg_s[ns], msb_s[ns], gTa_s[ns]

              outp_box = [None]

              def emit_po2(st):
                  # projection matmul for chunk ci, time-shared into the
                  # accumulator bank (idle between o_sb and the next po)
                  ci, o_sb, rbc = st
                  po2 = psacc.tile([128, CHUNK], f32, name="po2", tag="acc")
                  nc.tensor.matmul(
                      po2,
                      wo_sb[64:128, :],
                      o_sb[64:128, :],
                      start=True,
                      stop=True,
                      tile_position=(64, 0),
                  )
                  return po2

              def emit_tail(st, po2):
                  # normalize + residual for chunk ci (rbc broadcast has
                  # been in flight for a full chunk)
                  ci, o_sb, rbc = st
                  t1 = work.tile([128, CHUNK], f32, name="t1", tag="t1")
                  nc.vector.scalar_tensor_tensor(
                      t1, in0=po2, scalar=1.0, in1=rbc,
                      op0=ALU.mult, op1=ALU.mult,
                  )
                  if ci % 2 == 0:
                      outp_box[0] = outpool.tile(
                          [128, 2 * CHUNK], f32, name="outp", tag="out"
                      )
                  outp = outp_box[0]
                  half = outp[:, (ci % 2) * CHUNK : (ci % 2) * CHUNK + CHUNK]
                  nc.gpsimd.tensor_add(half, t1, Xf[:, ts(ci, CHUNK)])
                  if ci % 2 == 1:
                      # store via the GPSIMD SWDGE queue: keeps the SP
                      # HWDGE ring free for input loads
                      nc.gpsimd.dma_start(
                          out_d[:, bass.ds((ci - 1) * CHUNK, 2 * CHUNK)], outp
                      )

              def emit_sq(ci, q):
                  # one S^T quadrant-group: 4 K=128 matmuls from Msb
                  pair_a = pspair.tile([128, 1024], f32, name="pair_a", tag="pair")
                  pair_b = pspair.tile([128, 1024], f32, name="pair_b", tag="pair")
                  for j in range(4):
                      mi = 4 * q + j
                      dst = (pair_a if j < 2 else pair_b)[
                          :, (j % 2) * CHUNK : (j % 2) * CHUNK + CHUNK
                      ]
                      nc.tensor.matmul(
                          dst,
                          msb[:, mi * 128 : (mi + 1) * 128],
                          Xr[:, ts(ci, CHUNK)],
                          start=True,
                          stop=True,
                      )
                  return (pair_a, pair_b)

              def emit_exps(pairs, ets):
                  for pair in pairs:
                      et = etp.tile([128, 1024], fet, name="et", tag="et")
                      nc.scalar.activation(et, pair, ACTF.Exp)
                      ets.append(et)

              # Steady-state chunk: the exp stream paces everything; the
              # S^T matmuls depend only on Xr + Msb + pair-slot recycling
              # (3 slots -> a full exp of slack), so ACT never starves.
              # The po/po2/tail chain shares the accumulator bank and may
              # lag ~1 exp without touching the exp stream.
              pending = None
              pairs_q0 = emit_sq(0, 0)
              for ci in range(NCH):
                  ets = []
                  po2_prev = None
                  heat(H_PRE)
                  emit_exps(pairs_q0, ets)
                  if pending is not None:
                      po2_prev = emit_po2(pending)
                  if WEAVE and ci >= 1:
                      conv_mm(ci - 1, Xr_n, wgp_n, pg_n)
                  pairs_q1 = emit_sq(ci, 1)
                  emit_exps(pairs_q1, ets)
                  if pending is not None:
                      emit_tail(pending, po2_prev)
                      pending = None
                  heat(H_MID)

                  po = psacc.tile([128, CHUNK], f32, name="po", tag="acc")
                  for mi in range(4):
                      rhs = ets[mi // 2][:, (mi % 2) * CHUNK : (mi % 2) * CHUNK + CHUNK]
                      nc.tensor.matmul(
                          po, gTa_t[:, mi * 128 : (mi + 1) * 128], rhs,
                          start=(mi == 0), stop=False,
                      )
                  if ci + 1 < NCH:
                      pairs_q0 = emit_sq(ci + 1, 0)
                  for mi in range(4, 8):
                      rhs = ets[mi // 2][:, (mi % 2) * CHUNK : (mi % 2) * CHUNK + CHUNK]
                      nc.tensor.matmul(
                          po, gTa_t[:, mi * 128 : (mi + 1) * 128], rhs,
                          start=False, stop=(mi == 7),
                      )
                  if WEAVE and ci >= 1:
                      conv_tr(ci - 1, wt_n, pg_n, gTa_n, msb_n)
                  heat(H_PO)

                  # full-height po copy brings the s row (row 0) and o
                  # (rows 64:127) to SBUF; reciprocal of s and its Q7
                  # broadcast launch here so the tail streams next chunk
                  o_sb = work.tile([128, CHUNK], fmm, name="o_sb", tag="osb")
                  nc.vector.tensor_copy(o_sb, po)
                  rcp_f = work.tile([1, CHUNK], f32, name="rcp_f", tag="rcpf")
                  nc.vector.reciprocal_approx_fast(rcp_f, o_sb[0:1, :].bitcast(f32))
                  rbc = work.tile([128, CHUNK], f32, name="rbc", tag="rbc")
                  nc.gpsimd.partition_broadcast(rbc[:, :], rcp_f[0:1, :])

                  pending = (ci, o_sb, rbc)
              po2_last = emit_po2(pending)
              if WEAVE:
                  conv_step(NCH - 1, Xr_n, wt_n, wgp_n, pg_n, gTa_n, msb_n)
              emit_tail(pending, po2_last)

          # ---- prologue: first input + body-0 conv (serial, once) -------
          load_inputs(0)
          cvb_box[0] = psconv.tile([128, CHUNK], f32, name="cvb", tag="cv")
          for p in range(NCH):
              conv_step(p, xr_tiles[0], wcat_tiles[0][64:80, 0:128],
                        wcat_tiles[0][:, 128:208], pg_s[0], gTa_s[0], msb_s[0])
              heat(H_CONV)

          loop_cm = (
              tc.For_i(
                  0, reps // UNROLL, 1,
                  hint_engines=(
                      mybir.EngineType.PE,
                      mybir.EngineType.DVE,
                      mybir.EngineType.Activation,
                  ),
              )
              if reps > 1
              else nullcontext()
          )
          with loop_cm:
           for _s in range(1, UNROLL):
               load_inputs(_s)
           cvb_box[0] = psconv.tile([128, CHUNK], f32, name="cvb", tag="cv")
           for _u in range(UNROLL):
               if _u == UNROLL - 1 and UNROLL > 1:
                   # reload slot 0 BEFORE the last body so its woven conv
                   # (for next iteration's body 0) reads fresh data
                   load_inputs(0)
               body(_u)
           if UNROLL == 1 and reps > 1:
               # single-body loop: conv for the next iteration (slot 0)
               load_inputs(0)
               for p in range(NCH):
                   conv_step(p, xr_tiles[0], wcat_tiles[0][64:80, 0:128],
                             wcat_tiles[0][:, 128:208], pg_s[0], gTa_s[0],
                             msb_s[0])

    nc.compile()
    return nc


def _host_prep(x, W_theta, W_phi, W_g, W_o, gamma):
    B = np.asarray(x).shape[0]
    rnd = _round_fp32r if MM_MODE == "f32r" else (lambda a: np.asarray(a, np.float32))
    wcat = np.zeros((128, 336), dtype=np.float32)
    wcat[64:80, 0:128] = np.asarray(W_theta, np.float32)
    wcat[:, 128:192] = np.asarray(W_g, np.float32).T
    wcat[:, 192:208] = np.asarray(W_phi, np.float32).T
    wcat[64:128, 208:336] = float(gamma) * np.asarray(W_o, np.float32).T
    wcat = rnd(wcat)
    ident = np.eye(64, dtype=np.float32)
    xr = rnd(np.ascontiguousarray(np.asarray(x, dtype=np.float32)))
    in_maps = []
    for b in range(B):
        in_maps.append(
            {
                "xr": np.ascontiguousarray(xr[b].reshape(C, N)),
                "wcat": wcat,
                "ident": ident,
            }
        )
    return in_maps


def run(x, W_theta, W_phi, W_g, W_o, gamma, trace=False, **trace_kwargs):
    from concourse.bass_utils import run_bass_kernel_spmd

    nc = _build()
    in_maps = _host_prep(x, W_theta, W_phi, W_g, W_o, gamma)
    res = run_bass_kernel_spmd(
        nc, in_maps, core_ids=list(range(N_CORES)), trace=trace, **trace_kwargs
    )
    outs = [res.results[b]["out"].reshape(C, 64, 64) for b in range(N_CORES)]
    return np.stack(outs).astype(np.float32), res


def kernel(x, W_theta, W_phi, W_g, W_o, gamma):
    out, _ = run(x, W_theta, W_phi, W_g, W_o, gamma)
    return out
